# revision 1
# baseline (speedup 1.0000x reference)
"""Trainium2 Bass kernel for nn_Captioner_41412074668572 (retrieval_knn).

Computes: mean over (b, n) of min over l of ||image_features[b,n] - emb_table[token_ids[b,l]]||_2

Strategy (8 NeuronCores, data-parallel over batch B=32 -> 4 batches/core):
  host:   shard batches, gather embedding rows for each core's token_ids,
          lay out x^T in fp8 e4m3 and (-2*y)^T in bf16 (contraction dim on
          partitions; fp8 x measured at ~1e-5 final rel err vs 3e-7 for bf16,
          and halves the dominant DMA stream), precompute exact fp32 row
          norms x2, y2.
  device: d2[n,l] = x2[n] + y2[l] - 2*x.y  via PE matmul (fp8 x bf16 in,
          fp32 PSUM accumulate) producing -2*x.y; DVE adds the fp32 y2 bias
          and min-reduces over l (min commutes with +x2 and with monotone
          sqrt, so both are applied after the reduction); post: +x2, clamp,
          sqrt (+1 Newton step for the ACT spline's loose sqrt budget),
          row-sum -> [128,1] partials.
  host:   sum 8*[128] partials (float64), divide by B*N.
"""

import numpy as np
import ml_dtypes

B, N, L, D, V = 32, 2048, 128, 1024, 32000
N_CORES = 8
B_LOC = B // N_CORES          # 4 batches per core
P = 128                       # partitions
KC = D // P                   # 8 contraction chunks
NT = N // P                   # 16 n-tiles per batch
T = B_LOC * NT                # 64 tiles per core
X_DMA_SPLIT = 2               # split each x k-chunk DMA for queue parallelism

_CACHE: dict = {}

BF16 = ml_dtypes.bfloat16
FP8 = ml_dtypes.float8_e4m3


DEFAULT_KNOBS = dict(
    bufs_x=16,        # x tile slots (8 per batch in flight)
    x_split=2,        # DMA splits per x k-chunk
    dual_dma=True,    # alternate x DMAs between the two HWDGE engines
    sc_bufs=2,        # scratch bufs for the add/min pipeline
    y2_aug=False,     # fold y2 into the matmul as bf16 hi/lo K=2 rows
    y_contig=True,    # partition-major y layout -> contiguous y DMA
    small_on_pool=False,  # issue small y2b/x2t DMAs on gpsimd SWDGE
    x_fp8=True,       # x in fp8 e4m3 (empirical rel err ~1e-5 vs 3e-7 bf16)
    dve_batch=1,      # matmul tiles packed per PSUM bank / per DVE op (1 or 4)
)


def _build_nc(reps: int = 1, **knobs):
    """Build the Bass program. `reps` unrolls the whole body N times inside
    one NEFF (used only for marginal-time measurement in test.py)."""
    import concourse.tile as tile
    from concourse import bacc, mybir

    kn = dict(DEFAULT_KNOBS)
    kn.update(knobs)

    f32 = mybir.dt.float32
    bf16 = mybir.dt.bfloat16

    nc = bacc.Bacc("TRN2", target_bir_lowering=False, debug=False,
                   num_devices=N_CORES)

    xdt = mybir.dt.float8e4 if kn["x_fp8"] else bf16
    xname = "xt8" if kn["x_fp8"] else "xt"
    xt = nc.dram_tensor(xname, [B_LOC, KC, P, N], xdt, kind="ExternalInput")
    if kn["y_contig"]:
        ytc = nc.dram_tensor("ytc", [B_LOC, P, KC, L], bf16, kind="ExternalInput")
    else:
        yt = nc.dram_tensor("yt", [B_LOC, KC, P, L], bf16, kind="ExternalInput")
    x2t = nc.dram_tensor("x2t", [P, T], f32, kind="ExternalInput")
    out = nc.dram_tensor("out", [P, 1], f32, kind="ExternalOutput")
    if kn["y2_aug"]:
        yaux = nc.dram_tensor("yaux", [B_LOC, 2, L], bf16, kind="ExternalInput")
    else:
        y2b = nc.dram_tensor("y2b", [B_LOC, P, L], f32, kind="ExternalInput")

    with tile.TileContext(nc) as tc:
        with (
            tc.tile_pool(name="xp", bufs=kn["bufs_x"]) as xp,
            tc.tile_pool(name="yp", bufs=2) as yp,
            tc.tile_pool(name="y2p", bufs=2) as y2p,
            tc.tile_pool(name="cons", bufs=2) as cons,
            tc.tile_pool(name="sc", bufs=kn["sc_bufs"]) as scp,
            tc.tile_pool(name="ps", bufs=8, space="PSUM") as pp,
        ):
            x2s = cons.tile([P, T], f32, tag="x2s")
            small_eng = nc.gpsimd if kn["small_on_pool"] else nc.sync
            small_eng.dma_start(x2s[:], x2t[:])
            if kn["y2_aug"]:
                ones2 = cons.tile([2, P], bf16, tag="ones2")
                nc.gpsimd.memset(ones2[:], 1.0)

            def emit_body():
                mins = cons.tile([P, T], f32, tag="mins")
                for b in range(B_LOC):
                    xts = []
                    for k in range(KC):
                        xtile = xp.tile([P, N], xdt, tag="xt")
                        xs = kn["x_split"]
                        w = N // xs
                        for s in range(xs):
                            eng = nc.scalar if (kn["dual_dma"] and (k * xs + s) % 2) else nc.sync
                            eng.dma_start(xtile[:, s * w:(s + 1) * w],
                                          xt[b, k][:, s * w:(s + 1) * w])
                        xts.append(xtile)
                    ytile = yp.tile([P, KC, L], bf16, tag="yt")
                    if kn["y_contig"]:
                        nc.scalar.dma_start(ytile[:], ytc[b])
                    else:
                        nc.sync.dma_start(ytile[:], yt[b].rearrange("k p l -> p k l"))
                    G = kn["dve_batch"]
                    if kn["y2_aug"]:
                        yxs = y2p.tile([2, L], bf16, tag="yx")
                        nc.sync.dma_start(yxs[:], yaux[b])
                    elif G > 1:
                        # y2 replicated G-wide (via DMA, off the DVE)
                        y2w = y2p.tile([P, G, L], f32, tag="y2")
                        for j in range(G):
                            small_eng.dma_start(y2w[:, j, :], y2b[b])
                    else:
                        y2s = y2p.tile([P, L], f32, tag="y2")
                        small_eng.dma_start(y2s[:], y2b[b])

                    if G > 1:
                        # pack G n-tiles into one PSUM bank; one wide DVE
                        # add + one wide min-reduce per bank (amortizes the
                        # ~200ns/op DVE overhead that dominates at [128,128])
                        for g in range(NT // G):
                            ps4 = pp.tile([P, G, L], f32, tag="ps")
                            for j in range(G):
                                t = g * G + j
                                for k in range(KC):
                                    nc.tensor.matmul(
                                        ps4[:, j, :],
                                        xts[k][:, t * P:(t + 1) * P],
                                        ytile[:, k, :],
                                        start=(k == 0),
                                        stop=(k == KC - 1),
                                    )
                            sc4 = scp.tile([P, G, L], f32, tag="sc")
                            nc.vector.tensor_add(sc4[:], ps4[:], y2w[:])
                            col = b * NT + g * G
                            nc.vector.tensor_reduce(
                                mins[:, col:col + G], sc4[:],
                                axis=mybir.AxisListType.X, op=mybir.AluOpType.min,
                            )
                        continue

                    for t in range(NT):
                        ps = pp.tile([P, L], f32, tag="ps")
                        for k in range(KC):
                            nc.tensor.matmul(
                                ps[:],
                                xts[k][:, t * P:(t + 1) * P],  # lhsT [d, n=128]
                                ytile[:, k, :],                # rhs  [d, l=128]
                                start=(k == 0),
                                stop=(k == KC - 1) and not kn["y2_aug"],
                            )
                        col = b * NT + t
                        if kn["y2_aug"]:
                            # psum += ones.T @ [y2_hi; y2_lo] -> adds y2[l]
                            nc.tensor.matmul(ps[:], ones2[:], yxs[:],
                                             start=False, stop=True)
                            nc.vector.tensor_reduce(
                                mins[:, col:col + 1], ps[:],
                                axis=mybir.AxisListType.X, op=mybir.AluOpType.min,
                            )
                        else:
                            # sc = psum + y2 ; mins col = min_l(sc)
                            # (tensor_tensor_reduce would fuse these but fails
                            # on this HW path — CoreSim-only.)
                            sc = scp.tile([P, L], f32, tag="sc")
                            nc.vector.tensor_add(sc[:], ps[:], y2s[:])
                            nc.vector.tensor_reduce(
                                mins[:, col:col + 1], sc[:],
                                axis=mybir.AxisListType.X, op=mybir.AluOpType.min,
                            )

                # post: d2min = mins + x2 ; cost = sqrt(max(d2min, eps)) ; sum
                m2 = cons.tile([P, T], f32, tag="m2")
                nc.vector.tensor_add(m2[:], mins[:], x2s[:])
                nc.vector.tensor_scalar_max(m2[:], m2[:], 1e-20)
                s = cons.tile([P, T], f32, tag="s")
                nc.scalar.sqrt(s[:], m2[:])
                # one Newton step: s' = 0.5*(s + m2/s) (ACT sqrt table is loose)
                r = cons.tile([P, T], f32, tag="r")
                nc.vector.reciprocal(r[:], s[:])
                t2 = cons.tile([P, T], f32, tag="t2")
                nc.vector.tensor_mul(t2[:], m2[:], r[:])
                nc.vector.tensor_add(t2[:], t2[:], s[:])
                ov = cons.tile([P, 1], f32, tag="ov")
                nc.vector.reduce_sum(ov[:], t2[:], axis=mybir.AxisListType.X)
                nc.vector.tensor_scalar_mul(ov[:], ov[:], 0.5)
                nc.sync.dma_start(out[:], ov[:])

            for _ in range(reps):
                emit_body()

    nc.compile()
    return nc


def _get_nc(reps: int = 1, **knobs):
    key = ("nc", reps, tuple(sorted(knobs.items())))
    if key not in _CACHE:
        _CACHE[key] = _build_nc(reps, **knobs)
    return _CACHE[key]


def make_in_maps(image_features: np.ndarray, token_ids: np.ndarray,
                 emb_table: np.ndarray) -> list[dict]:
    """Shard + lay out the full inputs into per-core device input maps."""
    x = np.asarray(image_features, dtype=np.float32)
    tok = np.asarray(token_ids)
    emb = np.asarray(emb_table, dtype=np.float32)

    in_maps = []
    for c in range(N_CORES):
        xc = x[c * B_LOC:(c + 1) * B_LOC]                       # [4, N, D]
        # x^T: [b, k, p, n] = x[b, n, 128k+p]
        xT = np.ascontiguousarray(xc.transpose(0, 2, 1))        # [4, D, N]
        xT = xT.reshape(B_LOC, KC, P, N)
        xt_dev = xT.astype(BF16)
        xt8_dev = xT.astype(FP8)
        # exact fp32 row norms, laid out [p, b*NT+t] matching tile columns
        x2 = np.square(xc).sum(axis=-1, dtype=np.float64)       # [4, N]
        x2t_dev = np.ascontiguousarray(
            x2.reshape(B_LOC, NT, P).transpose(2, 0, 1).reshape(P, T)
        ).astype(np.float32)

        y = emb[tok[c * B_LOC:(c + 1) * B_LOC]]                 # [4, L, D]
        yT = np.ascontiguousarray((-2.0 * y).transpose(0, 2, 1))  # [4, D, L]
        yt_dev = yT.reshape(B_LOC, KC, P, L).astype(BF16)
        # partition-major variant: [b, p, k, l] = -2y[b, l, 128k+p]
        ytc_dev = np.ascontiguousarray(yt_dev.transpose(0, 2, 1, 3))  # [4, P, KC, L]
        y2 = np.square(y).sum(axis=-1, dtype=np.float64)        # [4, L]
        y2b_dev = np.ascontiguousarray(
            np.broadcast_to(y2[:, None, :].astype(np.float32), (B_LOC, P, L))
        )
        # y2 split into bf16 hi+lo rows (for the y2_aug matmul variant)
        y2f = y2.astype(np.float32)
        y2_hi = y2f.astype(BF16)
        y2_lo = (y2f - y2_hi.astype(np.float32)).astype(BF16)
        yaux_dev = np.ascontiguousarray(
            np.stack([y2_hi, y2_lo], axis=1))                   # [4, 2, L]

        in_maps.append({
            "xt": xt_dev,
            "xt8": xt8_dev,
            "yt": yt_dev,
            "ytc": ytc_dev,
            "y2b": y2b_dev,
            "yaux": yaux_dev,
            "x2t": x2t_dev,
        })
    return in_maps


def kernel(image_features: np.ndarray, token_ids: np.ndarray,
           emb_table: np.ndarray) -> np.ndarray:
    from concourse import mybir
    from concourse.bass_utils import run_bass_kernel_spmd

    nc = _get_nc()
    declared = {
        alloc.memorylocations[0].name
        for alloc in nc.m.functions[0].allocations
        if isinstance(alloc, mybir.MemoryLocationSet)
        and alloc.kind == "ExternalInput"
    }
    in_maps = [
        {k: v for k, v in m.items() if k in declared}
        for m in make_in_maps(image_features, token_ids, emb_table)
    ]
    res = run_bass_kernel_spmd(nc, in_maps, core_ids=list(range(N_CORES)))
    total = np.float64(0.0)
    for c in range(N_CORES):
        total += res.results[c]["out"].astype(np.float64).sum()
    return np.float32(total / (B * N))



# revision 2
# speedup vs baseline: 6.8106x; 6.8106x over previous
"""Trainium2 Bass kernel v2 for nn_Captioner_41412074668572 (retrieval_knn).

Computes: mean over (b, n) of min over l of ||image_features[b,n] - emb_table[token_ids[b,l]]||_2

v2 strategy (vs v1's out[n,l] / x-stationary / 512 weight loads):
  out[l, n] layout with y STATIONARY (32 weight loads/core instead of 512),
  x streams as the 512-wide moving operand -> matmul runs at the wide-free-dim
  production rate; fp8 DoubleRow (K=256/matmul) doubles PE throughput.

  d2[l, n] = -2*y.x (PE, fp8 DoubleRow) + x2[n] (PE, K=2 bf16 hi/lo aug matmul)
             + y2[l] (ACT per-partition bias, exact f32)
  cost[l, n] = sqrt(d2) on ACT (pre-min: sqrt is monotone, d2 ~ 1700 >> 0)
  min over l (partition axis) via DVE: 32x32 block transpose -> per-32-segment
  free-axis min -> 2 partition-fold min ops -> [32, 64] distances per batch
  -> row-sum -> acc[32, B_LOC] -> host sums and divides.

Sharding: data-parallel over batch B=32 -> 4 batches/core on 8 cores.
"""

import numpy as np
import ml_dtypes

B, N, L, D, V = 32, 2048, 128, 1024, 32000
N_CORES = 8
B_LOC = B // N_CORES          # 4 batches per core
P = 128                       # partitions
NCH = 4                       # 512-wide n-chunks per batch (PSUM bank width)
CW = N // NCH                 # 512

_CACHE: dict = {}

BF16 = ml_dtypes.bfloat16
FP8 = ml_dtypes.float8_e4m3


DEFAULT_KNOBS = dict(
    dr=True,          # fp8 DoubleRow (K=256/matmul); False -> plain K=128 (bf16 rate)
    x_split=1,        # DMA splits per x k-chunk
    fp16_sc=True,     # sqrt output fp16 (False -> bf16)
    y_eng="sync",     # engine for y/y2/x2a DMAs
    x_bufs=4,         # x tile buffering depth
    x_eng="sync",     # "sync"=all x on sync ring; "both"=alternate sync/scalar
    tail_eng="gpsimd",  # ring for tail fold DMAs (keeps HWDGE FIFOs clean)
    m1_bufs=2,        # m1all double buffering across reps
    aux_bufs=4,       # y/x2a/y2b tile buffering
    x_one=True,       # single 2MB x DMA per batch (partition-major DRAM layout)
)


def _build_nc(reps: int = 1, **knobs):
    import concourse.tile as tile
    from concourse import bacc, mybir

    kn = dict(DEFAULT_KNOBS)
    kn.update(knobs)

    f32 = mybir.dt.float32
    bf16 = mybir.dt.bfloat16
    fp16 = mybir.dt.float16 if kn["fp16_sc"] else mybir.dt.bfloat16
    fp8 = mybir.dt.float8e4

    DR = kn["dr"]
    KC = 4 if DR else 8       # contraction chunks (256 or 128 wide)
    KJ = 2 if DR else 1       # k-tiles per chunk (DoubleRow interleave)

    nc = bacc.Bacc("TRN2", target_bir_lowering=False, debug=False,
                   num_devices=N_CORES)

    # DRAM inputs (per-core shards, laid out by make_in_maps)
    if kn["x_one"]:
        x8 = nc.dram_tensor("x8", [B_LOC, P, KC, KJ, N], fp8, kind="ExternalInput")
    else:
        x8 = nc.dram_tensor("x8", [B_LOC, KC, P, KJ, N], fp8, kind="ExternalInput")
    y8 = nc.dram_tensor("y8", [B_LOC, P, KC, KJ, L], fp8, kind="ExternalInput")
    x2a = nc.dram_tensor("x2a", [B_LOC, 2, N], bf16, kind="ExternalInput")
    y2b = nc.dram_tensor("y2b", [B_LOC, P, 1], f32, kind="ExternalInput")
    out = nc.dram_tensor("out", [32, 1], f32, kind="ExternalOutput")

    with tile.TileContext(nc) as tc:
        with (
            tc.tile_pool(name="xp", bufs=2) as xp,
            tc.tile_pool(name="yp", bufs=2) as yp,
            tc.tile_pool(name="aux", bufs=2) as aux,
            tc.tile_pool(name="cons", bufs=1) as cons,
            tc.tile_pool(name="scp", bufs=2) as scp,
            tc.tile_pool(name="post", bufs=2) as post,
            tc.tile_pool(name="ps", bufs=2, space="PSUM") as pp,
        ):
            ones2 = cons.tile([2, P], bf16, tag="ones2")
            nc.gpsimd.memset(ones2[:], 1.0)
            y_eng = getattr(nc, kn["y_eng"])

            def emit_body():
                # per-batch 32-group mins land in m1all[:, b*64:(b+1)*64]
                m1all = cons.tile([P, B_LOC * 64], f32, tag="m1all",
                                  bufs=kn["m1_bufs"])
                for b in range(B_LOC):
                    # ---- DMAs ----
                    xt = xp.tile([P, KC, KJ, N], fp8, tag="x", bufs=kn["x_bufs"])
                    if kn["x_one"]:
                        nc.sync.dma_start(xt[:], x8[b])
                    else:
                        xs = kn["x_split"]
                        w = N // xs
                        for kc in range(KC):
                            for s in range(xs):
                                if kn["x_eng"] == "both" and (kc * xs + s) % 2:
                                    eng = nc.scalar
                                else:
                                    eng = nc.sync
                                eng.dma_start(xt[:, kc, :, s * w:(s + 1) * w],
                                              x8[b, kc][:, :, s * w:(s + 1) * w])
                    yt = yp.tile([P, KC, KJ, L], fp8, tag="y", bufs=kn["aux_bufs"])
                    y_eng.dma_start(yt[:], y8[b])
                    x2t = aux.tile([2, N], bf16, tag="x2", bufs=kn["aux_bufs"])
                    y_eng.dma_start(x2t[:], x2a[b])
                    y2t = aux.tile([P, 1], f32, tag="y2", bufs=kn["aux_bufs"])
                    y_eng.dma_start(y2t[:], y2b[b])

                    # ---- matmuls: ps[c] = -2*y.x + x2 ----
                    pss = [pp.tile([P, CW], f32, tag=f"ps{c}", name=f"ps{c}")[:]
                           for c in range(NCH)]
                    for kc in range(KC):
                        for c in range(NCH):
                            if DR:
                                nc.tensor.matmul(
                                    pss[c],
                                    yt[:, kc],                       # [128, 2, 128]
                                    xt[:, kc, :, c * CW:(c + 1) * CW],  # [128, 2, 512]
                                    start=(kc == 0), stop=False,
                                    perf_mode=mybir.MatmulPerfMode.DoubleRow,
                                )
                            else:
                                nc.tensor.matmul(
                                    pss[c],
                                    yt[:, kc, 0],                    # [128, 128]
                                    xt[:, kc, 0, c * CW:(c + 1) * CW],  # [128, 512]
                                    start=(kc == 0), stop=False,
                                )
                    for c in range(NCH):
                        nc.tensor.matmul(
                            pss[c], ones2[:],
                            x2t[:, c * CW:(c + 1) * CW],
                            start=False, stop=True,
                        )

                    # ---- ACT: sc = sqrt(ps + y2[l]) ----
                    sc = scp.tile([P, N], fp16, tag="sc")
                    for c in range(NCH):
                        nc.scalar.activation(
                            sc[:, c * CW:(c + 1) * CW], pss[c],
                            func=mybir.ActivationFunctionType.Sqrt,
                            bias=y2t[:], scale=1.0,
                        )

                    # ---- DVE: fused 32x32 block transpose + 32-seg min ----
                    nc.vector.tensor_reduce(
                        m1all[:, b * 64:(b + 1) * 64],
                        sc[:].rearrange("p (a b) -> p a b", b=32),
                        axis=mybir.AxisListType.X, op=mybir.AluOpType.min,
                        apply_transpose=True,
                    )

                # ---- tail: fold the 4 partition quadrants (l-blocks) ----
                # DVE can't mix partition bases, so shift halves via SBUF DMA.
                FW = B_LOC * 64
                t_eng = getattr(nc, kn["tail_eng"])
                m1b = post.tile([64, FW], f32, tag="m1b")
                t_eng.dma_start(m1b[:], m1all[64:128, :])
                f1 = post.tile([64, FW], f32, tag="f1")
                nc.vector.tensor_tensor(f1[:], m1all[0:64, :], m1b[:],
                                        op=mybir.AluOpType.min)
                f1b = post.tile([32, FW], f32, tag="f1b")
                t_eng.dma_start(f1b[:], f1[32:64, :])
                f2 = post.tile([32, FW], f32, tag="f2")
                nc.vector.tensor_tensor(f2[:], f1[0:32, :], f1b[:],
                                        op=mybir.AluOpType.min)
                ov = cons.tile([32, 1], f32, tag="ov", bufs=2)
                nc.vector.reduce_sum(ov[:], f2[:], axis=mybir.AxisListType.X)
                t_eng.dma_start(out[:], ov[:])

            for _ in range(reps):
                emit_body()

    nc.compile()
    return nc


def _get_nc(reps: int = 1, **knobs):
    key = ("nc", reps, tuple(sorted(knobs.items())))
    if key not in _CACHE:
        _CACHE[key] = _build_nc(reps, **knobs)
    return _CACHE[key]


def make_in_maps(image_features: np.ndarray, token_ids: np.ndarray,
                 emb_table: np.ndarray, **knobs) -> list[dict]:
    """Shard + lay out the full inputs into per-core device input maps."""
    kn = dict(DEFAULT_KNOBS)
    kn.update(knobs)
    DR = kn["dr"]
    KC = 4 if DR else 8
    KJ = 2 if DR else 1
    assert kn == dict(DEFAULT_KNOBS) or True

    x = np.asarray(image_features, dtype=np.float32)
    tok = np.asarray(token_ids)
    emb = np.asarray(emb_table, dtype=np.float32)

    in_maps = []
    for c in range(N_CORES):
        xc = x[c * B_LOC:(c + 1) * B_LOC]                       # [4, N, D]
        # x8[b, kc, p, j, n] = x[b, n, kc*(128*KJ) + j*128 + p]
        xT = np.ascontiguousarray(xc.transpose(0, 2, 1))        # [4, D, N]
        if kn["x_one"]:
            # [b, p, kc, j, n]
            x8_dev = np.ascontiguousarray(
                xT.reshape(B_LOC, KC, KJ, P, N).transpose(0, 3, 1, 2, 4)
            ).astype(FP8)
        else:
            x8_dev = np.ascontiguousarray(
                xT.reshape(B_LOC, KC, KJ, P, N).transpose(0, 1, 3, 2, 4)
            ).astype(FP8)

        x2 = np.square(xc).sum(axis=-1, dtype=np.float64).astype(np.float32)
        x2_hi = x2.astype(BF16)
        x2_lo = (x2 - x2_hi.astype(np.float32)).astype(BF16)
        x2a_dev = np.ascontiguousarray(np.stack([x2_hi, x2_lo], axis=1))  # [4,2,N]

        y = emb[tok[c * B_LOC:(c + 1) * B_LOC]]                 # [4, L, D]
        yT = np.ascontiguousarray((-2.0 * y).transpose(0, 2, 1))  # [4, D, L]
        # y8[b, p, kc, j, l] = -2y[b, l, kc*(128*KJ) + j*128 + p]
        y8_dev = np.ascontiguousarray(
            yT.reshape(B_LOC, KC, KJ, P, L).transpose(0, 3, 1, 2, 4)
        ).astype(FP8)

        y2 = np.square(y).sum(axis=-1, dtype=np.float64)        # [4, L]
        y2b_dev = np.ascontiguousarray(y2.astype(np.float32)[:, :, None])  # [4,128,1]

        in_maps.append({
            "x8": x8_dev,
            "y8": y8_dev,
            "x2a": x2a_dev,
            "y2b": y2b_dev,
        })
    return in_maps


def kernel(image_features: np.ndarray, token_ids: np.ndarray,
           emb_table: np.ndarray) -> np.ndarray:
    from concourse import mybir
    from concourse.bass_utils import run_bass_kernel_spmd

    nc = _get_nc()
    declared = {
        alloc.memorylocations[0].name
        for alloc in nc.m.functions[0].allocations
        if isinstance(alloc, mybir.MemoryLocationSet)
        and alloc.kind == "ExternalInput"
    }
    in_maps = [
        {k: v for k, v in m.items() if k in declared}
        for m in make_in_maps(image_features, token_ids, emb_table)
    ]
    res = run_bass_kernel_spmd(nc, in_maps, core_ids=list(range(N_CORES)))
    total = np.float64(0.0)
    for c in range(N_CORES):
        total += res.results[c]["out"].astype(np.float64).sum()
    return np.float32(total / (B * N))


# revision 3
# speedup vs baseline: 6.8168x; 1.0009x over previous
"""Trainium2 Bass kernel v2 for nn_Captioner_41412074668572 (retrieval_knn).

Computes: mean over (b, n) of min over l of ||image_features[b,n] - emb_table[token_ids[b,l]]||_2

v2 strategy (vs v1's out[n,l] / x-stationary / 512 weight loads):
  out[l, n] layout with y STATIONARY (32 weight loads/core instead of 512),
  x streams as the 512-wide moving operand -> matmul runs at the wide-free-dim
  production rate; fp8 DoubleRow (K=256/matmul) doubles PE throughput.

  d2[l, n] = -2*y.x (PE, fp8 DoubleRow) + x2[n] (PE, K=2 bf16 hi/lo aug matmul)
             + y2[l] (ACT per-partition bias, exact f32)
  cost[l, n] = sqrt(d2) on ACT (pre-min: sqrt is monotone, d2 ~ 1700 >> 0)
  min over l (partition axis) via DVE: 32x32 block transpose -> per-32-segment
  free-axis min -> 2 partition-fold min ops -> [32, 64] distances per batch
  -> row-sum -> acc[32, B_LOC] -> host sums and divides.

Sharding: data-parallel over batch B=32 -> 4 batches/core on 8 cores.
"""

import numpy as np
import ml_dtypes

B, N, L, D, V = 32, 2048, 128, 1024, 32000
N_CORES = 8
B_LOC = B // N_CORES          # 4 batches per core
P = 128                       # partitions
NCH = 4                       # 512-wide n-chunks per batch (PSUM bank width)
CW = N // NCH                 # 512

_CACHE: dict = {}

BF16 = ml_dtypes.bfloat16
FP8 = ml_dtypes.float8_e4m3


DEFAULT_KNOBS = dict(
    dr=True,          # fp8 DoubleRow (K=256/matmul); False -> plain K=128 (bf16 rate)
    x_split=1,        # DMA splits per x k-chunk
    fp16_sc=True,     # sqrt output fp16 (False -> bf16)
    y_eng="sync",     # engine for y/y2/x2a DMAs
    x_bufs=4,         # x tile buffering depth
    x_eng="sync",     # "sync"=all x on sync ring; "both"=alternate sync/scalar
    tail_eng="gpsimd",  # ring for tail fold DMAs (keeps HWDGE FIFOs clean)
    m1_bufs=2,        # m1all double buffering across reps
    aux_bufs=4,       # y/x2a/y2b tile buffering
    x_one=True,       # single 2MB x DMA per batch (partition-major DRAM layout)
    x2_fold="tail",   # "mm": K=2 aug matmul; "tail": x2+sqrt after the folds
)


def _build_nc(reps: int = 1, **knobs):
    import concourse.tile as tile
    from concourse import bacc, mybir

    kn = dict(DEFAULT_KNOBS)
    kn.update(knobs)

    f32 = mybir.dt.float32
    bf16 = mybir.dt.bfloat16
    fp16 = mybir.dt.float16 if kn["fp16_sc"] else mybir.dt.bfloat16
    fp8 = mybir.dt.float8e4

    DR = kn["dr"]
    KC = 4 if DR else 8       # contraction chunks (256 or 128 wide)
    KJ = 2 if DR else 1       # k-tiles per chunk (DoubleRow interleave)

    nc = bacc.Bacc("TRN2", target_bir_lowering=False, debug=False,
                   num_devices=N_CORES)

    # DRAM inputs (per-core shards, laid out by make_in_maps)
    if kn["x_one"]:
        x8 = nc.dram_tensor("x8", [B_LOC, P, KC, KJ, N], fp8, kind="ExternalInput")
    else:
        x8 = nc.dram_tensor("x8", [B_LOC, KC, P, KJ, N], fp8, kind="ExternalInput")
    y8 = nc.dram_tensor("y8", [B_LOC, P, KC, KJ, L], fp8, kind="ExternalInput")
    if kn["x2_fold"] == "mm":
        x2a = nc.dram_tensor("x2a", [B_LOC, 2, N], bf16, kind="ExternalInput")
    else:
        x2f = nc.dram_tensor("x2f", [32, B_LOC * 64], f32, kind="ExternalInput")
    y2b = nc.dram_tensor("y2b", [B_LOC, P, 1], f32, kind="ExternalInput")
    out = nc.dram_tensor("out", [32, 1], f32, kind="ExternalOutput")

    with tile.TileContext(nc) as tc:
        with (
            tc.tile_pool(name="xp", bufs=2) as xp,
            tc.tile_pool(name="yp", bufs=2) as yp,
            tc.tile_pool(name="aux", bufs=2) as aux,
            tc.tile_pool(name="cons", bufs=1) as cons,
            tc.tile_pool(name="scp", bufs=2) as scp,
            tc.tile_pool(name="post", bufs=2) as post,
            tc.tile_pool(name="ps", bufs=2, space="PSUM") as pp,
        ):
            if kn["x2_fold"] == "mm":
                ones2 = cons.tile([2, P], bf16, tag="ones2")
                nc.gpsimd.memset(ones2[:], 1.0)
            y_eng = getattr(nc, kn["y_eng"])

            def emit_batch(b, m1all):
                # ---- DMAs ----
                xt = xp.tile([P, KC, KJ, N], fp8, tag="x", bufs=kn["x_bufs"])
                if kn["x_one"]:
                    nc.sync.dma_start(xt[:], x8[b])
                else:
                    xs = kn["x_split"]
                    w = N // xs
                    for kc in range(KC):
                        for s in range(xs):
                            if kn["x_eng"] == "both" and (kc * xs + s) % 2:
                                eng = nc.scalar
                            else:
                                eng = nc.sync
                            eng.dma_start(xt[:, kc, :, s * w:(s + 1) * w],
                                          x8[b, kc][:, :, s * w:(s + 1) * w])
                yt = yp.tile([P, KC, KJ, L], fp8, tag="y", bufs=kn["aux_bufs"])
                y_eng.dma_start(yt[:], y8[b])
                if kn["x2_fold"] == "mm":
                    x2t = aux.tile([2, N], bf16, tag="x2", bufs=kn["aux_bufs"])
                    y_eng.dma_start(x2t[:], x2a[b])
                y2t = aux.tile([P, 1], f32, tag="y2", bufs=kn["aux_bufs"])
                y_eng.dma_start(y2t[:], y2b[b])

                # ---- matmuls: ps[c] = -2*y.x (+ x2) ----
                pss = [pp.tile([P, CW], f32, tag=f"ps{c}", name=f"ps{c}")[:]
                       for c in range(NCH)]
                for kc in range(KC):
                    for c in range(NCH):
                        if DR:
                            nc.tensor.matmul(
                                pss[c],
                                yt[:, kc],                       # [128, 2, 128]
                                xt[:, kc, :, c * CW:(c + 1) * CW],  # [128, 2, 512]
                                start=(kc == 0),
                                stop=(kc == KC - 1) and kn["x2_fold"] != "mm",
                                perf_mode=mybir.MatmulPerfMode.DoubleRow,
                            )
                        else:
                            nc.tensor.matmul(
                                pss[c],
                                yt[:, kc, 0],                    # [128, 128]
                                xt[:, kc, 0, c * CW:(c + 1) * CW],  # [128, 512]
                                start=(kc == 0),
                                stop=(kc == KC - 1) and kn["x2_fold"] != "mm",
                            )
                if kn["x2_fold"] == "mm":
                    for c in range(NCH):
                        nc.tensor.matmul(
                            pss[c], ones2[:],
                            x2t[:, c * CW:(c + 1) * CW],
                            start=False, stop=True,
                        )

                # ---- ACT: sc = sqrt(ps+y2) (mm) or ps+y2-1024 (tail) ----
                sc = scp.tile([P, N], fp16, tag="sc")
                act_fn = (mybir.ActivationFunctionType.Sqrt
                          if kn["x2_fold"] == "mm"
                          else mybir.ActivationFunctionType.Identity)
                for c in range(NCH):
                    nc.scalar.activation(
                        sc[:, c * CW:(c + 1) * CW], pss[c],
                        func=act_fn,
                        bias=y2t[:], scale=1.0,
                    )

                # ---- DVE: fused 32x32 block transpose + 32-seg min ----
                nc.vector.tensor_reduce(
                    m1all[:, b * 64:(b + 1) * 64],
                    sc[:].rearrange("p (a b) -> p a b", b=32),
                    axis=mybir.AxisListType.X, op=mybir.AluOpType.min,
                    apply_transpose=True,
                )

            def emit_tail(m1all):
                # fold the 4 partition quadrants (l-blocks); DVE can't mix
                # partition bases, so shift halves via SBUF DMA.
                FW = B_LOC * 64
                t_eng = getattr(nc, kn["tail_eng"])
                m1b = post.tile([64, FW], f32, tag="m1b")
                t_eng.dma_start(m1b[:], m1all[64:128, :])
                f1 = post.tile([64, FW], f32, tag="f1")
                nc.vector.tensor_tensor(f1[:], m1all[0:64, :], m1b[:],
                                        op=mybir.AluOpType.min)
                f1b = post.tile([32, FW], f32, tag="f1b")
                t_eng.dma_start(f1b[:], f1[32:64, :])
                f2 = post.tile([32, FW], f32, tag="f2")
                nc.vector.tensor_tensor(f2[:], f1[0:32, :], f1b[:],
                                        op=mybir.AluOpType.min)
                if kn["x2_fold"] != "mm":
                    # f2 holds min(-2xy + y2) - 1024; add x2+1024, sqrt, sum
                    x2ft = aux.tile([32, FW], f32, tag="x2f", bufs=2)
                    y_eng.dma_start(x2ft[:], x2f[:])
                    m2 = post.tile([32, FW], f32, tag="m2")
                    nc.vector.tensor_add(m2[:], f2[:], x2ft[:])
                    sq = post.tile([32, FW], f32, tag="sq")
                    nc.scalar.sqrt(sq[:], m2[:])
                    f2 = sq
                ov = cons.tile([32, 1], f32, tag="ov", bufs=2)
                nc.vector.reduce_sum(ov[:], f2[:], axis=mybir.AxisListType.X)
                t_eng.dma_start(out[:], ov[:])

            # software-pipelined: rep r's tail is emitted after rep r+1's
            # first batch so its chain never blocks the ACT/DVE rings at
            # the rep boundary.
            pending = None
            for _ in range(reps):
                m1all = cons.tile([P, B_LOC * 64], f32, tag="m1all",
                                  bufs=kn["m1_bufs"], name="m1all")
                for b in range(B_LOC):
                    emit_batch(b, m1all)
                    if b == 0 and pending is not None:
                        emit_tail(pending)
                        pending = None
                pending = m1all
            emit_tail(pending)

    nc.compile()
    return nc


def _get_nc(reps: int = 1, **knobs):
    key = ("nc", reps, tuple(sorted(knobs.items())))
    if key not in _CACHE:
        _CACHE[key] = _build_nc(reps, **knobs)
    return _CACHE[key]


def make_in_maps(image_features: np.ndarray, token_ids: np.ndarray,
                 emb_table: np.ndarray, **knobs) -> list[dict]:
    """Shard + lay out the full inputs into per-core device input maps."""
    kn = dict(DEFAULT_KNOBS)
    kn.update(knobs)
    DR = kn["dr"]
    KC = 4 if DR else 8
    KJ = 2 if DR else 1
    assert kn == dict(DEFAULT_KNOBS) or True

    x = np.asarray(image_features, dtype=np.float32)
    tok = np.asarray(token_ids)
    emb = np.asarray(emb_table, dtype=np.float32)

    in_maps = []
    for c in range(N_CORES):
        xc = x[c * B_LOC:(c + 1) * B_LOC]                       # [4, N, D]
        # x8[b, kc, p, j, n] = x[b, n, kc*(128*KJ) + j*128 + p]
        xT = np.ascontiguousarray(xc.transpose(0, 2, 1))        # [4, D, N]
        if kn["x_one"]:
            # [b, p, kc, j, n]
            x8_dev = np.ascontiguousarray(
                xT.reshape(B_LOC, KC, KJ, P, N).transpose(0, 3, 1, 2, 4)
            ).astype(FP8)
        else:
            x8_dev = np.ascontiguousarray(
                xT.reshape(B_LOC, KC, KJ, P, N).transpose(0, 1, 3, 2, 4)
            ).astype(FP8)

        x2 = np.square(xc).sum(axis=-1, dtype=np.float64).astype(np.float32)
        x2_hi = x2.astype(BF16)
        x2_lo = (x2 - x2_hi.astype(np.float32)).astype(BF16)
        x2a_dev = np.ascontiguousarray(np.stack([x2_hi, x2_lo], axis=1))  # [4,2,N]
        # tail layout: x2f[i, b*64 + j] = x2[b, 32*j + i] + 1024 (centering undo)
        x2f_dev = np.ascontiguousarray(
            (x2 + 1024.0).reshape(B_LOC, 64, 32).transpose(2, 0, 1)
            .reshape(32, B_LOC * 64))

        y = emb[tok[c * B_LOC:(c + 1) * B_LOC]]                 # [4, L, D]
        yT = np.ascontiguousarray((-2.0 * y).transpose(0, 2, 1))  # [4, D, L]
        # y8[b, p, kc, j, l] = -2y[b, l, kc*(128*KJ) + j*128 + p]
        y8_dev = np.ascontiguousarray(
            yT.reshape(B_LOC, KC, KJ, P, L).transpose(0, 3, 1, 2, 4)
        ).astype(FP8)

        y2 = np.square(y).sum(axis=-1, dtype=np.float64)        # [4, L]
        y2off = 0.0 if kn["x2_fold"] == "mm" else 1024.0
        y2b_dev = np.ascontiguousarray(
            (y2 - y2off).astype(np.float32)[:, :, None])        # [4,128,1]

        in_maps.append({
            "x8": x8_dev,
            "y8": y8_dev,
            "x2a": x2a_dev,
            "x2f": x2f_dev,
            "y2b": y2b_dev,
        })
    return in_maps


def kernel(image_features: np.ndarray, token_ids: np.ndarray,
           emb_table: np.ndarray) -> np.ndarray:
    from concourse import mybir
    from concourse.bass_utils import run_bass_kernel_spmd

    nc = _get_nc()
    declared = {
        alloc.memorylocations[0].name
        for alloc in nc.m.functions[0].allocations
        if isinstance(alloc, mybir.MemoryLocationSet)
        and alloc.kind == "ExternalInput"
    }
    in_maps = [
        {k: v for k, v in m.items() if k in declared}
        for m in make_in_maps(image_features, token_ids, emb_table)
    ]
    res = run_bass_kernel_spmd(nc, in_maps, core_ids=list(range(N_CORES)))
    total = np.float64(0.0)
    for c in range(N_CORES):
        total += res.results[c]["out"].astype(np.float64).sum()
    return np.float32(total / (B * N))


# revision 5
# speedup vs baseline: 7.3532x; 1.0787x over previous
"""Trainium2 Bass kernel v2 for nn_Captioner_41412074668572 (retrieval_knn).

Computes: mean over (b, n) of min over l of ||image_features[b,n] - emb_table[token_ids[b,l]]||_2

v2 strategy (vs v1's out[n,l] / x-stationary / 512 weight loads):
  out[l, n] layout with y STATIONARY (32 weight loads/core instead of 512),
  x streams as the 512-wide moving operand -> matmul runs at the wide-free-dim
  production rate; fp8 DoubleRow (K=256/matmul) doubles PE throughput.

  sc[l, n]   = -2*y.x (PE, fp8 DoubleRow, K=256/MM, moving x 512-wide)
               + (y2[l] - 1024) (ACT per-partition bias, exact f32) -> fp16
  min over l (partition axis) via DVE fused op: 32x32 block transpose +
  per-32-segment min in ONE TensorReduce(apply_transpose=True) -> m1all
  [128, 64/batch]; tail (software-pipelined into the NEXT rep so it never
  blocks the ACT/DVE FIFOs): 2 partition-quadrant folds (SBUF-DMA shifts +
  DVE min), + x2[n]+1024, ACT sqrt, row-sum -> out[32, 1]; host sums/divides.

  sqrt AFTER min here (d2-domain min): min commutes with the monotone sqrt;
  fp16 on (d2 - 1024 - x2) ~ +-150 keeps min-selection noise ~0.03.

DMA: x is 8MB/core fp8 (the wall: ~21 GB/s/SDMA-engine x 16 with both NCs
per HBM stack active); one 2MB partition-major DMA per batch on the sync
HWDGE ring only (a waiting trigger blocks the whole ring FIFO, so the ring
carries nothing that waits on compute); y/y2 batched once per rep; tail
DMAs on gpsimd SWDGE.

Sharding: data-parallel over batch B=32 -> 4 batches/core on 8 cores.
"""

import numpy as np
import ml_dtypes

B, N, L, D, V = 32, 2048, 128, 1024, 32000
N_CORES = 8
B_LOC = B // N_CORES          # 4 batches per core
P = 128                       # partitions
NCH = 4                       # 512-wide n-chunks per batch (PSUM bank width)
CW = N // NCH                 # 512

_CACHE: dict = {}

BF16 = ml_dtypes.bfloat16
FP8 = ml_dtypes.float8_e4m3


DEFAULT_KNOBS = dict(
    dr=True,          # fp8 DoubleRow (K=256/matmul); False -> plain K=128 (bf16 rate)
    x_split=1,        # DMA splits per x k-chunk
    fp16_sc=True,     # sqrt output fp16 (False -> bf16)
    y_eng="sync",     # engine for y/y2/x2a DMAs
    x_bufs=4,         # x tile buffering depth
    x_eng="sync",     # "sync"=all x on sync ring; "both"=alternate sync/scalar
    tail_eng="gpsimd",  # ring for tail fold DMAs (keeps HWDGE FIFOs clean)
    m1_bufs=2,        # m1all double buffering across reps
    aux_bufs=4,       # y/x2a/y2b tile buffering
    x_one=True,       # single 2MB x DMA per batch (partition-major DRAM layout)
    x2_fold="tail",   # "mm": K=2 aug matmul; "tail": x2+sqrt after the folds
)


def _build_nc(reps: int = 1, **knobs):
    import concourse.tile as tile
    from concourse import bacc, mybir

    kn = dict(DEFAULT_KNOBS)
    kn.update(knobs)

    f32 = mybir.dt.float32
    bf16 = mybir.dt.bfloat16
    fp16 = mybir.dt.float16 if kn["fp16_sc"] else mybir.dt.bfloat16
    fp8 = mybir.dt.float8e4

    DR = kn["dr"]
    KC = 4 if DR else 8       # contraction chunks (256 or 128 wide)
    KJ = 2 if DR else 1       # k-tiles per chunk (DoubleRow interleave)

    nc = bacc.Bacc("TRN2", target_bir_lowering=False, debug=False,
                   num_devices=N_CORES)

    # DRAM inputs (per-core shards, laid out by make_in_maps)
    if kn["x_one"]:
        x8 = nc.dram_tensor("x8", [B_LOC, P, KC, KJ, N], fp8, kind="ExternalInput")
    else:
        x8 = nc.dram_tensor("x8", [B_LOC, KC, P, KJ, N], fp8, kind="ExternalInput")
    y8 = nc.dram_tensor("y8", [P, B_LOC, KC, KJ, L], fp8, kind="ExternalInput")
    if kn["x2_fold"] == "mm":
        x2a = nc.dram_tensor("x2a", [B_LOC, 2, N], bf16, kind="ExternalInput")
    else:
        x2f = nc.dram_tensor("x2f", [32, B_LOC * 64], f32, kind="ExternalInput")
    y2b = nc.dram_tensor("y2b", [P, B_LOC], f32, kind="ExternalInput")
    out = nc.dram_tensor("out", [32, 1], f32, kind="ExternalOutput")

    with tile.TileContext(nc) as tc:
        with (
            tc.tile_pool(name="xp", bufs=2) as xp,
            tc.tile_pool(name="yp", bufs=2) as yp,
            tc.tile_pool(name="aux", bufs=2) as aux,
            tc.tile_pool(name="cons", bufs=1) as cons,
            tc.tile_pool(name="scp", bufs=2) as scp,
            tc.tile_pool(name="post", bufs=2) as post,
            tc.tile_pool(name="ps", bufs=2, space="PSUM") as pp,
        ):
            if kn["x2_fold"] == "mm":
                ones2 = cons.tile([2, P], bf16, tag="ones2")
                nc.gpsimd.memset(ones2[:], 1.0)
            y_eng = getattr(nc, kn["y_eng"])

            def emit_rep_head():
                # per-rep loads: all 4 batches' y + y2 in one DMA each
                yta = yp.tile([P, B_LOC, KC, KJ, L], fp8, tag="y", bufs=2)
                y_eng.dma_start(yta[:], y8[:])
                y2ta = aux.tile([P, B_LOC], f32, tag="y2", bufs=2)
                y_eng.dma_start(y2ta[:], y2b[:])
                return yta, y2ta

            def emit_batch(b, m1all, yta, y2ta):
                yt = yta[:, b]
                y2t = y2ta[:, b:b + 1]
                # ---- DMAs ----
                xt = xp.tile([P, KC, KJ, N], fp8, tag="x", bufs=kn["x_bufs"])
                if kn["x_one"]:
                    nc.sync.dma_start(xt[:], x8[b])
                else:
                    xs = kn["x_split"]
                    w = N // xs
                    for kc in range(KC):
                        for s in range(xs):
                            if kn["x_eng"] == "both" and (kc * xs + s) % 2:
                                eng = nc.scalar
                            else:
                                eng = nc.sync
                            eng.dma_start(xt[:, kc, :, s * w:(s + 1) * w],
                                          x8[b, kc][:, :, s * w:(s + 1) * w])
                if kn["x2_fold"] == "mm":
                    x2t = aux.tile([2, N], bf16, tag="x2", bufs=kn["aux_bufs"])
                    y_eng.dma_start(x2t[:], x2a[b])

                # ---- matmuls: ps[c] = -2*y.x (+ x2) ----
                pss = [pp.tile([P, CW], f32, tag=f"ps{c}", name=f"ps{c}")[:]
                       for c in range(NCH)]
                for kc in range(KC):
                    for c in range(NCH):
                        if DR:
                            nc.tensor.matmul(
                                pss[c],
                                yt[:, kc],                       # [128, 2, 128]
                                xt[:, kc, :, c * CW:(c + 1) * CW],  # [128, 2, 512]
                                start=(kc == 0),
                                stop=(kc == KC - 1) and kn["x2_fold"] != "mm",
                                perf_mode=mybir.MatmulPerfMode.DoubleRow,
                            )
                        else:
                            nc.tensor.matmul(
                                pss[c],
                                yt[:, kc, 0],                    # [128, 128]
                                xt[:, kc, 0, c * CW:(c + 1) * CW],  # [128, 512]
                                start=(kc == 0),
                                stop=(kc == KC - 1) and kn["x2_fold"] != "mm",
                            )
                if kn["x2_fold"] == "mm":
                    for c in range(NCH):
                        nc.tensor.matmul(
                            pss[c], ones2[:],
                            x2t[:, c * CW:(c + 1) * CW],
                            start=False, stop=True,
                        )

                # ---- ACT: sc = sqrt(ps+y2) (mm) or ps+y2-1024 (tail) ----
                sc = scp.tile([P, N], fp16, tag="sc")
                act_fn = (mybir.ActivationFunctionType.Sqrt
                          if kn["x2_fold"] == "mm"
                          else mybir.ActivationFunctionType.Identity)
                for c in range(NCH):
                    nc.scalar.activation(
                        sc[:, c * CW:(c + 1) * CW], pss[c],
                        func=act_fn,
                        bias=y2t[:], scale=1.0,
                    )

                # ---- DVE: fused 32x32 block transpose + 32-seg min ----
                nc.vector.tensor_reduce(
                    m1all[:, b * 64:(b + 1) * 64],
                    sc[:].rearrange("p (a b) -> p a b", b=32),
                    axis=mybir.AxisListType.X, op=mybir.AluOpType.min,
                    apply_transpose=True,
                )

            def emit_tail(m1all):
                # fold the 4 partition quadrants (l-blocks); DVE can't mix
                # partition bases, so shift halves via SBUF DMA.
                FW = B_LOC * 64
                t_eng = getattr(nc, kn["tail_eng"])
                m1b = post.tile([64, FW], f32, tag="m1b")
                t_eng.dma_start(m1b[:], m1all[64:128, :])
                f1 = post.tile([64, FW], f32, tag="f1")
                nc.vector.tensor_tensor(f1[:], m1all[0:64, :], m1b[:],
                                        op=mybir.AluOpType.min)
                f1b = post.tile([32, FW], f32, tag="f1b")
                t_eng.dma_start(f1b[:], f1[32:64, :])
                f2 = post.tile([32, FW], f32, tag="f2")
                nc.vector.tensor_tensor(f2[:], f1[0:32, :], f1b[:],
                                        op=mybir.AluOpType.min)
                if kn["x2_fold"] != "mm":
                    # f2 holds min(-2xy + y2) - 1024; add x2+1024, sqrt, sum
                    x2ft = aux.tile([32, FW], f32, tag="x2f", bufs=2)
                    y_eng.dma_start(x2ft[:], x2f[:])
                    m2 = post.tile([32, FW], f32, tag="m2")
                    nc.vector.tensor_add(m2[:], f2[:], x2ft[:])
                    sq = post.tile([32, FW], f32, tag="sq")
                    nc.scalar.sqrt(sq[:], m2[:])
                    f2 = sq
                ov = cons.tile([32, 1], f32, tag="ov", bufs=2)
                nc.vector.reduce_sum(ov[:], f2[:], axis=mybir.AxisListType.X)
                t_eng.dma_start(out[:], ov[:])

            # software-pipelined: rep r's tail is emitted after rep r+1's
            # first batch so its chain never blocks the ACT/DVE rings at
            # the rep boundary.
            pending = None
            for _ in range(reps):
                m1all = cons.tile([P, B_LOC * 64], f32, tag="m1all",
                                  bufs=kn["m1_bufs"], name="m1all")
                yta, y2ta = emit_rep_head()
                for b in range(B_LOC):
                    emit_batch(b, m1all, yta, y2ta)
                    if b == 0 and pending is not None:
                        emit_tail(pending)
                        pending = None
                pending = m1all
            emit_tail(pending)

    nc.compile()
    return nc


def _get_nc(reps: int = 1, **knobs):
    key = ("nc", reps, tuple(sorted(knobs.items())))
    if key not in _CACHE:
        _CACHE[key] = _build_nc(reps, **knobs)
    return _CACHE[key]


def make_in_maps(image_features: np.ndarray, token_ids: np.ndarray,
                 emb_table: np.ndarray, **knobs) -> list[dict]:
    """Shard + lay out the full inputs into per-core device input maps."""
    kn = dict(DEFAULT_KNOBS)
    kn.update(knobs)
    DR = kn["dr"]
    KC = 4 if DR else 8
    KJ = 2 if DR else 1
    assert kn == dict(DEFAULT_KNOBS) or True

    x = np.asarray(image_features, dtype=np.float32)
    tok = np.asarray(token_ids)
    emb = np.asarray(emb_table, dtype=np.float32)

    in_maps = []
    for c in range(N_CORES):
        xc = x[c * B_LOC:(c + 1) * B_LOC]                       # [4, N, D]
        # x8[b, kc, p, j, n] = x[b, n, kc*(128*KJ) + j*128 + p]
        xT = np.ascontiguousarray(xc.transpose(0, 2, 1))        # [4, D, N]
        if kn["x_one"]:
            # [b, p, kc, j, n]
            x8_dev = np.ascontiguousarray(
                xT.reshape(B_LOC, KC, KJ, P, N).transpose(0, 3, 1, 2, 4)
            ).astype(FP8)
        else:
            x8_dev = np.ascontiguousarray(
                xT.reshape(B_LOC, KC, KJ, P, N).transpose(0, 1, 3, 2, 4)
            ).astype(FP8)

        x2 = np.square(xc).sum(axis=-1, dtype=np.float64).astype(np.float32)
        x2_hi = x2.astype(BF16)
        x2_lo = (x2 - x2_hi.astype(np.float32)).astype(BF16)
        x2a_dev = np.ascontiguousarray(np.stack([x2_hi, x2_lo], axis=1))  # [4,2,N]
        # tail layout: x2f[i, b*64 + j] = x2[b, 32*j + i] + 1024 (centering undo)
        x2f_dev = np.ascontiguousarray(
            (x2 + 1024.0).reshape(B_LOC, 64, 32).transpose(2, 0, 1)
            .reshape(32, B_LOC * 64))

        y = emb[tok[c * B_LOC:(c + 1) * B_LOC]]                 # [4, L, D]
        yT = np.ascontiguousarray((-2.0 * y).transpose(0, 2, 1))  # [4, D, L]
        # y8[p, b, kc, j, l] = -2y[b, l, kc*(128*KJ) + j*128 + p]
        y8_dev = np.ascontiguousarray(
            yT.reshape(B_LOC, KC, KJ, P, L).transpose(3, 0, 1, 2, 4)
        ).astype(FP8)

        y2 = np.square(y).sum(axis=-1, dtype=np.float64)        # [4, L]
        y2off = 0.0 if kn["x2_fold"] == "mm" else 1024.0
        y2b_dev = np.ascontiguousarray(
            (y2 - y2off).astype(np.float32).T)                  # [128, 4]

        in_maps.append({
            "x8": x8_dev,
            "y8": y8_dev,
            "x2a": x2a_dev,
            "x2f": x2f_dev,
            "y2b": y2b_dev,
        })
    return in_maps


def kernel(image_features: np.ndarray, token_ids: np.ndarray,
           emb_table: np.ndarray) -> np.ndarray:
    from concourse import mybir
    from concourse.bass_utils import run_bass_kernel_spmd

    nc = _get_nc()
    declared = {
        alloc.memorylocations[0].name
        for alloc in nc.m.functions[0].allocations
        if isinstance(alloc, mybir.MemoryLocationSet)
        and alloc.kind == "ExternalInput"
    }
    in_maps = [
        {k: v for k, v in m.items() if k in declared}
        for m in make_in_maps(image_features, token_ids, emb_table)
    ]
    res = run_bass_kernel_spmd(nc, in_maps, core_ids=list(range(N_CORES)))
    total = np.float64(0.0)
    for c in range(N_CORES):
        total += res.results[c]["out"].astype(np.float64).sum()
    return np.float32(total / (B * N))
